# revision 19
# baseline (speedup 1.0000x reference)
"""ConvDualAttention Trainium2 kernel (Bass/Tile), 8-core data-parallel.

Contract: kernel(**inputs) takes the FULL unsharded inputs, shards batch b
across the 8 NeuronCores (one batch per core), and returns the full
(8, 128, 4096) float32 output.

Math (per batch b, per head h, D=128, X=4096):
  y_p   = dwconv3(x) + t_p/s_p           (p in q,k,v; BN folded so that
                                          W_eff_p @ y_p == pw_p @ BN(conv))
  k     = W_eff_k @ y_k ; sk = softmax(k over d)
  kat   = SCALE * q^T @ sk               (SCALE folded into W_q)
  gout  = GW @ q + gb ; sig = sigmoid(gout)
  out_h = v @ kat + sig^T * v
  out   = out_w @ merge(out_h) + out_b

Kernel factorizations (validated against the jax reference):
  * q is never materialized: kat_h = wtq_h^T @ R_h with
    R_h = y_q^T^T ... i.e. R[c,(h,d)] = sum_x y_q[c,x] sk'[x,(h,d)],
    where y_q INCLUDES the conv bias t'_q, so the rank-1 bias/sigma
    correction of the baseline is unnecessary.  y_qT is produced from
    y_q by DMA transpose (bf16), not by extra PE work.
  * v@kat through the output projection collapses to W3 @ y_v with
    W3 = sum_h outw_h @ (Wv_h^T @ kat_h)^T, computed on-chip from the
    tiny per-head kat matrices.
  * everything flows in bf16 (PSUM accumulation in fp32); final output
    is fp32.
"""
import numpy as np
import ml_dtypes

import concourse.bass as bass
import concourse.tile as tile
from concourse import bacc, mybir
from concourse.bass_utils import run_bass_kernel_spmd

F32 = mybir.dt.float32
BF16 = mybir.dt.bfloat16
AF = mybir.ActivationFunctionType
ALU = mybir.AluOpType

B = 8
DIM = 128
HEADS = 8
INNER = DIM * HEADS
X = 4096
EPS = 1e-5
SCALE = DIM ** -0.5
NT = X // 128          # 32 x-tiles of 128
NCH = X // 512         # 8 chunks of 512
NCB = X // 1024        # 4 chunks of 1024

_NC = None
TRACE = False
LAST_EXEC_NS = None


def _bf(a):
    return np.ascontiguousarray(np.asarray(a, np.float32).astype(ml_dtypes.bfloat16))


def _prep(inputs):
    """Host-side weight folding. Returns dict of DRAM input arrays."""
    f = lambda k: np.asarray(inputs[k], np.float32)
    wt = {}
    tprime = {}
    diag_cols = []
    for p in ("q", "k", "v"):
        s = f(p + "_g") / np.sqrt(f(p + "_v") + EPS)        # (128,)
        t = f(p + "_b") - f(p + "_m") * s
        tprime[p] = t / s
        w_eff = f(p + "_pw") * s[None, :]                    # (1024, 128)
        wt[p] = np.ascontiguousarray(w_eff.T)                # (128, 1024)
        dw = f(p + "_dw")[:, 0, :]                           # (128, 3)
        for j in range(3):
            diag_cols.append(np.diag(dw[:, j]).astype(np.float32))
    s_gt = f("gt_g") / np.sqrt(f("gt_v") + EPS)
    t_gt = f("gt_b") - f("gt_m") * s_gt
    gw = f("gt_pw") * (f("gt_dw")[:, 0, 0] * s_gt)[None, :]  # (128, 128)
    gb = f("gt_pw") @ t_gt                                   # (128,)
    w_eff_q = wt["q"].T                                      # (1024, 128)
    gqt = np.concatenate(
        [(gw @ w_eff_q[h * 128:(h + 1) * 128, :]).T for h in range(HEADS)], axis=1
    )                                                        # (128 i, 1024 h*o)
    out_w = f("out_w")                                       # (128, 1024)
    outwt = np.concatenate(
        [np.ascontiguousarray(out_w[:, h * 128:(h + 1) * 128].T) for h in range(HEADS)],
        axis=1,
    )                                                        # (128 d, 1024 h*o)
    wvdm = np.concatenate(
        [wt["v"].T[h * 128:(h + 1) * 128, :] for h in range(HEADS)], axis=1
    )                                                        # (128 d, 1024 h*i)
    diag = np.concatenate(diag_cols, axis=1)                 # (128, 1152)
    wtq_s = wt["q"] * SCALE                                  # (128 i, 1024 d)
    biasp = np.stack(
        [tprime["q"], tprime["k"], tprime["v"], gb, f("out_b")], axis=1
    )                                                        # (128, 5)
    return {
        "wtk": _bf(wt["k"]),
        "wtv": _bf(wt["v"]),
        "gqt": _bf(gqt),
        "outwt": _bf(outwt),
        "wvdm": _bf(wvdm),
        "diag": _bf(diag),
        "biasp": np.ascontiguousarray(biasp.astype(np.float32)),
        "wtqr": _bf(wtq_s),
        "ident": _bf(np.eye(128, dtype=np.float32)),
    }


def _build():
    nc = bacc.Bacc("TRN2", target_bir_lowering=False, debug=False, num_devices=B)
    xb_d = nc.dram_tensor("xb", [128, X + 2], BF16, kind="ExternalInput").ap()
    wtk_d = nc.dram_tensor("wtk", [128, INNER], BF16, kind="ExternalInput").ap()
    wtv_d = nc.dram_tensor("wtv", [128, INNER], BF16, kind="ExternalInput").ap()
    gqt_d = nc.dram_tensor("gqt", [128, INNER], BF16, kind="ExternalInput").ap()
    outwt_d = nc.dram_tensor("outwt", [128, INNER], BF16, kind="ExternalInput").ap()
    wvdm_d = nc.dram_tensor("wvdm", [128, INNER], BF16, kind="ExternalInput").ap()
    diag_d = nc.dram_tensor("diag", [128, 9 * 128], BF16, kind="ExternalInput").ap()
    biasp_d = nc.dram_tensor("biasp", [128, 5], F32, kind="ExternalInput").ap()
    wtqr_d = nc.dram_tensor("wtqr", [128, INNER], BF16, kind="ExternalInput").ap()
    ident_d = nc.dram_tensor("ident", [128, 128], BF16, kind="ExternalInput").ap()
    out_d = nc.dram_tensor("out", [128, X], F32, kind="ExternalOutput").ap()

    # host biasp column order: q, k, v, gb, out_b
    BQ, BK, BV, BG, BO = 0, 1, 2, 3, 4
    NS = NT // 2   # 16 super-tiles of 2 x-tiles (2048 K columns)

    with tile.TileContext(nc) as tc:
        with (
            tc.tile_pool(name="const", bufs=1) as cp,
            tc.tile_pool(name="sigp", bufs=3) as sigp,
        ):
            wtk = cp.tile([128, INNER], BF16)
            wtv = cp.tile([128, INNER], BF16)
            gqt = cp.tile([128, INNER], BF16)
            outwt = cp.tile([128, INNER], BF16)
            wvdm = cp.tile([128, INNER], BF16)
            wtqr = cp.tile([128, INNER], BF16)
            diag = cp.tile([128, 9 * 128], BF16)
            biasp = cp.tile([128, 5], F32)
            yq = cp.tile([128, X], BF16, tag="yq")
            yk = cp.tile([128, X], BF16, tag="yk")
            yv = cp.tile([128, X], BF16, tag="yv")
            yqt = cp.tile([128, X], BF16, tag="yqt")
            sksb = cp.tile([128, NT * 1024], BF16, tag="sksb")
            zt = cp.tile([128, NT * 8], F32, tag="zt")
            zi = cp.tile([128, NT * 8], F32, tag="zi")
            zib = cp.tile([128, NT * 8], BF16, tag="zib")
            r_sb = cp.tile([128, INNER], BF16, tag="rsb")
            kat_sb = cp.tile([128, INNER], BF16, tag="katsb")
            m2_sb = cp.tile([128, INNER], BF16, tag="m2sb")
            w3t_sb = cp.tile([128, 128], BF16, tag="w3t")

            xb = cp.tile([128, X + 2], BF16, tag="xb")
            ident = cp.tile([128, 128], BF16, tag="ident")
            # split the big input DMA + spread weights over both HWDGE queues
            for q4 in range(4):
                lo = q4 * 1024
                hi = min(X + 2, lo + 1026)
                nc.sync.dma_start(out=xb[:, lo:hi], in_=xb_d[:, lo:hi])
            nc.sync.dma_start(out=diag, in_=diag_d)
            nc.scalar.dma_start(out=biasp, in_=biasp_d)
            nc.scalar.dma_start(out=ident, in_=ident_d)
            for i, (sb_t, dr) in enumerate((
                    (wtk, wtk_d), (wtqr, wtqr_d), (wvdm, wvdm_d),
                    (outwt, outwt_d), (gqt, gqt_d), (wtv, wtv_d))):
                eng = nc.sync if i % 2 == 0 else nc.scalar
                eng.dma_start(out=sb_t, in_=dr)

            ys = {"q": yq, "k": yk, "v": yv}
            bcol = {"q": BQ, "k": BK, "v": BV}
            dbase = {"q": 0, "k": 3, "v": 6}

            # ---- y-stage: depthwise conv via 3 shifted diagonal matmuls ----
            with (
                tc.tile_pool(name="yps", bufs=2, space="PSUM") as yps,
                tc.tile_pool(name="tps", bufs=2, space="PSUM") as tps,
            ):
                for p in ("k", "q", "v"):
                    for c in range(NCB):
                        pt = yps.tile([128, 1024], F32, tag="yps")
                        for j in range(3):
                            dsl = diag[:, (dbase[p] + j) * 128:(dbase[p] + j + 1) * 128]
                            for u in range(2):
                                nc.tensor.matmul(
                                    pt[:, u * 512:(u + 1) * 512], dsl,
                                    xb[:, c * 1024 + u * 512 + j:
                                       c * 1024 + u * 512 + j + 512],
                                    start=(j == 0), stop=(j == 2),
                                    skip_group_check=True,
                                )
                        osl = slice(c * 1024, (c + 1) * 1024)
                        if p == "v":
                            nc.vector.tensor_scalar(
                                ys[p][:, osl], pt, biasp[:, BV:BV + 1], None,
                                ALU.add,
                            )
                        else:
                            nc.scalar.activation(
                                ys[p][:, osl], pt,
                                AF.Identity, bias=biasp[:, bcol[p]:bcol[p] + 1],
                            )

                # transpose all yq tiles on PE (bf16 PSUM out),
                # 4 tiles per PSUM buffer, evac on DVE
                for half in range(8):
                    tp = tps.tile([128, 512], BF16, tag="tp")
                    for q4 in range(4):
                        t = half * 4 + q4
                        nc.tensor.transpose(
                            tp[:, q4 * 128:(q4 + 1) * 128],
                            yq[:, t * 128:(t + 1) * 128],
                            ident,
                        )
                    t0 = half * 4 * 128
                    nc.vector.tensor_copy(yqt[:, t0:t0 + 512], tp)

            # ---- phase A: K -> exp -> z -> normalize -> R ----
            with (
                tc.tile_pool(name="rps", bufs=1, space="PSUM") as rps,
                tc.tile_pool(name="kqps", bufs=2, space="PSUM") as kqps,
            ):
                r_ps = rps.tile([128, 1024], F32, tag="r")
                for t in range(NT):
                    kt = kqps.tile([128, 1024], F32, tag="kq")
                    ykt = yk[:, t * 128:(t + 1) * 128]
                    for u in range(2):
                        nc.tensor.matmul(
                            kt[:, u * 512:(u + 1) * 512],
                            ykt, wtk[:, u * 512:(u + 1) * 512],
                            start=True, stop=True,
                        )
                    nc.scalar.activation(
                        sksb[:, t * 1024:(t + 1) * 1024], kt, AF.Exp,
                    )
                    if t % 4 == 3:
                        z0 = (t - 3) * 8
                        # z: per-head row sums of exp(k), 4 tiles per op
                        nc.vector.tensor_reduce(
                            zt[:, z0:z0 + 32],
                            sksb[:, (t - 3) * 1024:(t + 1) * 1024].rearrange(
                                "p (t h d) -> p t h d", t=4, h=8
                            ),
                            mybir.AxisListType.X, ALU.add,
                        )
                        nc.vector.reciprocal(
                            zi[:, z0:z0 + 32], zt[:, z0:z0 + 32]
                        )
                        nc.vector.tensor_copy(
                            zib[:, z0:z0 + 32], zi[:, z0:z0 + 32]
                        )
                for t in range(NT):
                    # normalize sk in place on GpSimd (broadcast per head)
                    skv = sksb[:, t * 1024:(t + 1) * 1024].rearrange(
                        "p (h d) -> p h d", h=8
                    )
                    zb = zib[:, t * 8:(t + 1) * 8][:, :, None
                             ].to_broadcast((128, 8, 128))
                    nc.gpsimd.tensor_tensor(skv, skv, zb, ALU.mult)
                    yqtt = yqt[:, t * 128:(t + 1) * 128]
                    nc.tensor.matmul(
                        r_ps[:, 0:512], yqtt,
                        sksb[:, t * 1024:t * 1024 + 512],
                        start=(t == 0), stop=(t == NT - 1),
                        skip_group_check=True,
                    )
                    nc.tensor.matmul(
                        r_ps[:, 512:1024], yqtt,
                        sksb[:, t * 1024 + 512:(t + 1) * 1024],
                        start=(t == 0), stop=(t == NT - 1),
                        skip_group_check=True,
                    )
                nc.scalar.copy(r_sb, r_ps)

            # kat -> M2 -> W3T per head (tiny matmul chain)
            with (
                tc.tile_pool(name="smps", bufs=1, space="PSUM") as smps,
            ):
                w3t_ps = smps.tile([128, 128], F32, tag="w3tp")
                for h in range(HEADS):
                    hsl = slice(h * 128, (h + 1) * 128)
                    kat_ps = smps.tile([128, 128], F32, tag="katp")
                    nc.tensor.matmul(
                        kat_ps, wtqr[:, hsl], r_sb[:, hsl],
                        start=True, stop=True, skip_group_check=True,
                    )
                    nc.scalar.copy(kat_sb[:, hsl], kat_ps)
                    m2_ps = smps.tile([128, 128], F32, tag="m2p")
                    nc.tensor.matmul(
                        m2_ps, kat_sb[:, hsl], wvdm[:, hsl],
                        start=True, stop=True, skip_group_check=True,
                    )
                    nc.scalar.copy(m2_sb[:, hsl], m2_ps)
                    nc.tensor.matmul(
                        w3t_ps, m2_sb[:, hsl], outwt[:, hsl],
                        start=(h == 0), stop=(h == HEADS - 1),
                        skip_group_check=True,
                    )
                nc.scalar.copy(w3t_sb, w3t_ps)

            # ---- phase B: gate + final projection per 512-chunk ----
            with (
                tc.tile_pool(name="goutps", bufs=2, space="PSUM") as goutps,
                tc.tile_pool(name="vps", bufs=2, space="PSUM") as vps,
                tc.tile_pool(name="finps", bufs=2, space="PSUM") as finps,
                tc.tile_pool(name="bpool", bufs=3) as bp,
            ):
                for c in range(NCH):
                    csl = slice(c * 512, (c + 1) * 512)
                    fin_ps = finps.tile([128, 512], F32, tag="fin")
                    nc.tensor.matmul(
                        fin_ps, w3t_sb, yv[:, csl],
                        start=True, stop=False, skip_group_check=True,
                    )
                    for hp in range(HEADS // 2):
                        g_ps = goutps.tile([128, 1024], F32, tag="gout")
                        for d in range(2):
                            h = hp * 2 + d
                            nc.tensor.matmul(
                                g_ps[:, d * 512:(d + 1) * 512],
                                gqt[:, h * 128:(h + 1) * 128],
                                yq[:, csl], start=True, stop=True,
                            )
                        sig = sigp.tile([128, 1024], BF16, tag="sig")
                        nc.scalar.activation(
                            sig, g_ps, AF.Sigmoid, bias=biasp[:, BG:BG + 1],
                        )
                        for d in range(2):
                            h = hp * 2 + d
                            v_ps = vps.tile([128, 512], F32, tag="vp")
                            nc.tensor.matmul(
                                v_ps, wtv[:, h * 128:(h + 1) * 128],
                                yv[:, csl], start=True, stop=True,
                            )
                            gate = bp.tile([128, 512], BF16, tag="gate")
                            nc.vector.tensor_tensor(
                                gate, v_ps, sig[:, d * 512:(d + 1) * 512],
                                ALU.mult,
                            )
                            nc.tensor.matmul(
                                fin_ps, outwt[:, h * 128:(h + 1) * 128], gate,
                                start=False, stop=(h == HEADS - 1),
                                skip_group_check=True,
                            )
                    fin_sb = bp.tile([128, 512], F32, tag="finsb")
                    nc.scalar.activation(
                        fin_sb, fin_ps, AF.Identity, bias=biasp[:, BO:BO + 1]
                    )
                    eng = nc.sync if c % 2 == 0 else nc.scalar
                    eng.dma_start(out=out_d[:, csl], in_=fin_sb)

    nc.compile()
    return nc


def kernel(**inputs):
    global _NC, LAST_EXEC_NS
    host = _prep(inputs)
    if _NC is None:
        _NC = _build()
    x = np.asarray(inputs["x"], np.float32)
    in_maps = []
    for b in range(B):
        xp = np.pad(x[b], ((0, 0), (1, 1)))
        m = {"xb": _bf(xp)}
        m.update(host)
        in_maps.append(m)
    res = run_bass_kernel_spmd(
        _NC, in_maps, core_ids=list(range(B)), trace=TRACE
    )
    LAST_EXEC_NS = res.exec_time_ns
    return np.stack([r["out"] for r in res.results]).astype(np.float32)


# revision 20
# speedup vs baseline: 1.3897x; 1.3897x over previous
"""ConvDualAttention Trainium2 kernel (Bass/Tile), 8-core data-parallel.

Contract: kernel(**inputs) takes the FULL unsharded inputs, shards batch b
across the 8 NeuronCores (one batch per core), and returns the full
(8, 128, 4096) float32 output.

Math (per batch b, per head h, D=128, X=4096):
  y_p   = dwconv3(x) + t_p/s_p           (p in q,k,v; BN folded so that
                                          W_eff_p @ y_p == pw_p @ BN(conv))
  k     = W_eff_k @ y_k ; sk = softmax(k over d)
  kat   = SCALE * q^T @ sk               (SCALE folded into W_q)
  gout  = GW @ q + gb ; sig = sigmoid(gout)
  out_h = v @ kat + sig^T * v
  out   = out_w @ merge(out_h) + out_b

Kernel factorizations (validated against the jax reference):
  * q is never materialized: kat_h = wtq_h^T @ R_h with
    R_h = y_q^T^T ... i.e. R[c,(h,d)] = sum_x y_q[c,x] sk'[x,(h,d)],
    where y_q INCLUDES the conv bias t'_q, so the rank-1 bias/sigma
    correction of the baseline is unnecessary.  y_qT is produced from
    y_q by DMA transpose (bf16), not by extra PE work.
  * v@kat through the output projection collapses to W3 @ y_v with
    W3 = sum_h outw_h @ (Wv_h^T @ kat_h)^T, computed on-chip from the
    tiny per-head kat matrices.
  * everything flows in bf16 (PSUM accumulation in fp32); final output
    is fp32.
"""
import numpy as np
import ml_dtypes

import concourse.bass as bass
import concourse.tile as tile
from concourse import bacc, mybir
from concourse.bass_utils import run_bass_kernel_spmd

F32 = mybir.dt.float32
BF16 = mybir.dt.bfloat16
AF = mybir.ActivationFunctionType
ALU = mybir.AluOpType

B = 8
DIM = 128
HEADS = 8
INNER = DIM * HEADS
X = 4096
EPS = 1e-5
SCALE = DIM ** -0.5
NT = X // 128          # 32 x-tiles of 128
NCH = X // 512         # 8 chunks of 512
NCB = X // 1024        # 4 chunks of 1024

_NC = None
TRACE = False
LAST_EXEC_NS = None


def _bf(a):
    return np.ascontiguousarray(np.asarray(a, np.float32).astype(ml_dtypes.bfloat16))


def _prep(inputs):
    """Host-side weight folding. Returns dict of DRAM input arrays."""
    f = lambda k: np.asarray(inputs[k], np.float32)
    wt = {}
    tprime = {}
    diag_cols = []
    for p in ("q", "k", "v"):
        s = f(p + "_g") / np.sqrt(f(p + "_v") + EPS)        # (128,)
        t = f(p + "_b") - f(p + "_m") * s
        tprime[p] = t / s
        w_eff = f(p + "_pw") * s[None, :]                    # (1024, 128)
        wt[p] = np.ascontiguousarray(w_eff.T)                # (128, 1024)
        dw = f(p + "_dw")[:, 0, :]                           # (128, 3)
        for j in range(3):
            diag_cols.append(np.diag(dw[:, j]).astype(np.float32))
    s_gt = f("gt_g") / np.sqrt(f("gt_v") + EPS)
    t_gt = f("gt_b") - f("gt_m") * s_gt
    gw = f("gt_pw") * (f("gt_dw")[:, 0, 0] * s_gt)[None, :]  # (128, 128)
    gb = f("gt_pw") @ t_gt                                   # (128,)
    w_eff_q = wt["q"].T                                      # (1024, 128)
    gqt = np.concatenate(
        [(gw @ w_eff_q[h * 128:(h + 1) * 128, :]).T for h in range(HEADS)], axis=1
    )                                                        # (128 i, 1024 h*o)
    out_w = f("out_w")                                       # (128, 1024)
    outwt = np.concatenate(
        [np.ascontiguousarray(out_w[:, h * 128:(h + 1) * 128].T) for h in range(HEADS)],
        axis=1,
    )                                                        # (128 d, 1024 h*o)
    wvdm = np.concatenate(
        [wt["v"].T[h * 128:(h + 1) * 128, :] for h in range(HEADS)], axis=1
    )                                                        # (128 d, 1024 h*i)
    diag = np.concatenate(diag_cols, axis=1)                 # (128, 1152)
    wtq_s = wt["q"] * SCALE                                  # (128 i, 1024 d)
    biasp = np.stack(
        [tprime["q"], tprime["k"], tprime["v"], gb, f("out_b")], axis=1
    )                                                        # (128, 5)
    return {
        "wtk": _bf(wt["k"]),
        "wtv": _bf(wt["v"]),
        "gqt": _bf(gqt),
        "outwt": _bf(outwt),
        "wvdm": _bf(wvdm),
        "diag": _bf(diag),
        "biasp": np.ascontiguousarray(biasp.astype(np.float32)),
        "wtqr": _bf(wtq_s),
        "ident": _bf(np.eye(128, dtype=np.float32)),
    }


def _build():
    nc = bacc.Bacc("TRN2", target_bir_lowering=False, debug=False, num_devices=B)
    xb_d = nc.dram_tensor("xb", [128, X + 2], BF16, kind="ExternalInput").ap()
    wtk_d = nc.dram_tensor("wtk", [128, INNER], BF16, kind="ExternalInput").ap()
    wtv_d = nc.dram_tensor("wtv", [128, INNER], BF16, kind="ExternalInput").ap()
    gqt_d = nc.dram_tensor("gqt", [128, INNER], BF16, kind="ExternalInput").ap()
    outwt_d = nc.dram_tensor("outwt", [128, INNER], BF16, kind="ExternalInput").ap()
    wvdm_d = nc.dram_tensor("wvdm", [128, INNER], BF16, kind="ExternalInput").ap()
    diag_d = nc.dram_tensor("diag", [128, 9 * 128], BF16, kind="ExternalInput").ap()
    biasp_d = nc.dram_tensor("biasp", [128, 5], F32, kind="ExternalInput").ap()
    wtqr_d = nc.dram_tensor("wtqr", [128, INNER], BF16, kind="ExternalInput").ap()
    ident_d = nc.dram_tensor("ident", [128, 128], BF16, kind="ExternalInput").ap()
    out_d = nc.dram_tensor("out", [128, X], F32, kind="ExternalOutput").ap()

    # host biasp column order: q, k, v, gb, out_b
    BQ, BK, BV, BG, BO = 0, 1, 2, 3, 4
    NS = NT // 2   # 16 super-tiles of 2 x-tiles (2048 K columns)

    with tile.TileContext(nc) as tc:
        with (
            tc.tile_pool(name="const", bufs=1) as cp,
            tc.tile_pool(name="sigp", bufs=3) as sigp,
        ):
            wtk = cp.tile([128, INNER], BF16)
            wtv = cp.tile([128, INNER], BF16)
            gqt = cp.tile([128, INNER], BF16)
            outwt = cp.tile([128, INNER], BF16)
            wvdm = cp.tile([128, INNER], BF16)
            wtqr = cp.tile([128, INNER], BF16)
            diag = cp.tile([128, 9 * 128], BF16)
            biasp = cp.tile([128, 5], F32)
            yq = cp.tile([128, X], BF16, tag="yq")
            yk = cp.tile([128, X], BF16, tag="yk")
            yv = cp.tile([128, X], BF16, tag="yv")
            yqt = cp.tile([128, X], BF16, tag="yqt")
            sksb = cp.tile([128, NT * 1024], BF16, tag="sksb")
            zt = cp.tile([128, NT * 8], F32, tag="zt")
            zi = cp.tile([128, NT * 8], F32, tag="zi")
            zib = cp.tile([128, NT * 8], BF16, tag="zib")
            r_sb = cp.tile([128, INNER], BF16, tag="rsb")
            kat_sb = cp.tile([128, INNER], BF16, tag="katsb")
            m2_sb = cp.tile([128, INNER], BF16, tag="m2sb")
            w3t_sb = cp.tile([128, 128], BF16, tag="w3t")

            xb = cp.tile([128, X + 2], BF16, tag="xb")
            ident = cp.tile([128, 128], BF16, tag="ident")
            nc.sync.dma_start(out=xb, in_=xb_d)
            nc.sync.dma_start(out=diag, in_=diag_d)
            nc.sync.dma_start(out=biasp, in_=biasp_d)
            nc.sync.dma_start(out=ident, in_=ident_d)
            for sb_t, dr in ((wtk, wtk_d), (wtqr, wtqr_d), (wvdm, wvdm_d),
                             (outwt, outwt_d), (gqt, gqt_d), (wtv, wtv_d)):
                nc.sync.dma_start(out=sb_t, in_=dr)

            ys = {"q": yq, "k": yk, "v": yv}
            bcol = {"q": BQ, "k": BK, "v": BV}
            dbase = {"q": 0, "k": 3, "v": 6}

            # ---- y-stage: depthwise conv via 3 shifted diagonal matmuls ----
            with (
                tc.tile_pool(name="yps", bufs=2, space="PSUM") as yps,
            ):
                for p in ("k", "q", "v"):
                    for c in range(NCB):
                        pt = yps.tile([128, 1024], F32, tag="yps")
                        for j in range(3):
                            dsl = diag[:, (dbase[p] + j) * 128:(dbase[p] + j + 1) * 128]
                            for u in range(2):
                                nc.tensor.matmul(
                                    pt[:, u * 512:(u + 1) * 512], dsl,
                                    xb[:, c * 1024 + u * 512 + j:
                                       c * 1024 + u * 512 + j + 512],
                                    start=(j == 0), stop=(j == 2),
                                    skip_group_check=True,
                                )
                        osl = slice(c * 1024, (c + 1) * 1024)
                        nc.scalar.activation(
                            ys[p][:, osl], pt,
                            AF.Identity, bias=biasp[:, bcol[p]:bcol[p] + 1],
                        )
                        if p == "q":
                            for tt in range(8):
                                t = c * 8 + tt
                                nc.sync.dma_start_transpose(
                                    yqt[:, t * 128:(t + 1) * 128],
                                    yq[:, t * 128:(t + 1) * 128],
                                )
            # ---- phase A: K -> exp -> z -> normalize -> R ----
            with (
                tc.tile_pool(name="rps", bufs=1, space="PSUM") as rps,
                tc.tile_pool(name="kqps", bufs=2, space="PSUM") as kqps,
            ):
                r_ps = rps.tile([128, 1024], F32, tag="r")
                for t in range(NT):
                    kt = kqps.tile([128, 1024], F32, tag="kq")
                    ykt = yk[:, t * 128:(t + 1) * 128]
                    for u in range(2):
                        nc.tensor.matmul(
                            kt[:, u * 512:(u + 1) * 512],
                            ykt, wtk[:, u * 512:(u + 1) * 512],
                            start=True, stop=True,
                        )
                    nc.scalar.activation(
                        sksb[:, t * 1024:(t + 1) * 1024], kt, AF.Exp,
                    )
                    nc.vector.tensor_reduce(
                        zt[:, t * 8:(t + 1) * 8],
                        sksb[:, t * 1024:(t + 1) * 1024].rearrange(
                            "p (h d) -> p h d", h=8
                        ),
                        mybir.AxisListType.X, ALU.add,
                    )
                    if t % 4 == 3:
                        z0 = (t - 3) * 8
                        nc.vector.reciprocal(
                            zi[:, z0:z0 + 32], zt[:, z0:z0 + 32]
                        )
                        nc.vector.tensor_copy(
                            zib[:, z0:z0 + 32], zi[:, z0:z0 + 32]
                        )
                for t in range(NT):
                    # normalize sk in place on GpSimd (broadcast per head)
                    skv = sksb[:, t * 1024:(t + 1) * 1024].rearrange(
                        "p (h d) -> p h d", h=8
                    )
                    zb = zib[:, t * 8:(t + 1) * 8][:, :, None
                             ].to_broadcast((128, 8, 128))
                    nc.gpsimd.tensor_tensor(skv, skv, zb, ALU.mult)
                    yqtt = yqt[:, t * 128:(t + 1) * 128]
                    nc.tensor.matmul(
                        r_ps[:, 0:512], yqtt,
                        sksb[:, t * 1024:t * 1024 + 512],
                        start=(t == 0), stop=(t == NT - 1),
                        skip_group_check=True,
                    )
                    nc.tensor.matmul(
                        r_ps[:, 512:1024], yqtt,
                        sksb[:, t * 1024 + 512:(t + 1) * 1024],
                        start=(t == 0), stop=(t == NT - 1),
                        skip_group_check=True,
                    )
                nc.vector.tensor_copy(r_sb, r_ps)

            # kat -> M2 -> W3T per head (tiny matmul chain)
            with (
                tc.tile_pool(name="smps", bufs=2, space="PSUM") as smps,
            ):
                w3t_ps = smps.tile([128, 128], F32, tag="w3tp")
                for h in range(HEADS):
                    hsl = slice(h * 128, (h + 1) * 128)
                    kat_ps = smps.tile([128, 128], F32, tag="katp")
                    nc.tensor.matmul(
                        kat_ps, wtqr[:, hsl], r_sb[:, hsl],
                        start=True, stop=True, skip_group_check=True,
                    )
                    nc.vector.tensor_copy(kat_sb[:, hsl], kat_ps)
                    m2_ps = smps.tile([128, 128], F32, tag="m2p")
                    nc.tensor.matmul(
                        m2_ps, kat_sb[:, hsl], wvdm[:, hsl],
                        start=True, stop=True, skip_group_check=True,
                    )
                    nc.vector.tensor_copy(m2_sb[:, hsl], m2_ps)
                    nc.tensor.matmul(
                        w3t_ps, m2_sb[:, hsl], outwt[:, hsl],
                        start=(h == 0), stop=(h == HEADS - 1),
                        skip_group_check=True,
                    )
                nc.vector.tensor_copy(w3t_sb, w3t_ps)

            # ---- phase B: gate + final projection per 512-chunk ----
            with (
                tc.tile_pool(name="goutps", bufs=2, space="PSUM") as goutps,
                tc.tile_pool(name="vps", bufs=2, space="PSUM") as vps,
                tc.tile_pool(name="finps", bufs=2, space="PSUM") as finps,
                tc.tile_pool(name="bpool", bufs=3) as bp,
            ):
                for c in range(NCH):
                    csl = slice(c * 512, (c + 1) * 512)
                    fin_ps = finps.tile([128, 512], F32, tag="fin")
                    for hp in range(HEADS // 2):
                        g_ps = goutps.tile([128, 1024], F32, tag="gout")
                        for d in range(2):
                            h = hp * 2 + d
                            nc.tensor.matmul(
                                g_ps[:, d * 512:(d + 1) * 512],
                                gqt[:, h * 128:(h + 1) * 128],
                                yq[:, csl], start=True, stop=True,
                            )
                        sig = sigp.tile([128, 1024], BF16, tag="sig")
                        nc.scalar.activation(
                            sig, g_ps, AF.Sigmoid, bias=biasp[:, BG:BG + 1],
                        )
                        for d in range(2):
                            h = hp * 2 + d
                            v_ps = vps.tile([128, 512], F32, tag="vp")
                            nc.tensor.matmul(
                                v_ps, wtv[:, h * 128:(h + 1) * 128],
                                yv[:, csl], start=True, stop=True,
                            )
                            gate = bp.tile([128, 512], BF16, tag="gate")
                            nc.vector.tensor_tensor(
                                gate, v_ps, sig[:, d * 512:(d + 1) * 512],
                                ALU.mult,
                            )
                            nc.tensor.matmul(
                                fin_ps, outwt[:, h * 128:(h + 1) * 128], gate,
                                start=(h == 0), stop=False,
                                skip_group_check=True,
                            )
                    nc.tensor.matmul(
                        fin_ps, w3t_sb, yv[:, csl],
                        start=False, stop=True, skip_group_check=True,
                    )
                    fin_sb = bp.tile([128, 512], F32, tag="finsb")
                    nc.scalar.activation(
                        fin_sb, fin_ps, AF.Identity, bias=biasp[:, BO:BO + 1]
                    )
                    nc.sync.dma_start(out=out_d[:, csl], in_=fin_sb)

    nc.compile()
    return nc


def kernel(**inputs):
    global _NC, LAST_EXEC_NS
    host = _prep(inputs)
    if _NC is None:
        _NC = _build()
    x = np.asarray(inputs["x"], np.float32)
    in_maps = []
    for b in range(B):
        xp = np.pad(x[b], ((0, 0), (1, 1)))
        m = {"xb": _bf(xp)}
        m.update(host)
        in_maps.append(m)
    res = run_bass_kernel_spmd(
        _NC, in_maps, core_ids=list(range(B)), trace=TRACE
    )
    LAST_EXEC_NS = res.exec_time_ns
    return np.stack([r["out"] for r in res.results]).astype(np.float32)
